# revision 14
# baseline (speedup 1.0000x reference)
"""Trainium2 Bass kernel for the EnhancedBCMLayer (block-circulant matrix layer).

Math: out[B, 16f+i] = sum_{g,j} iv[f,g,(i-j)%16] * x[B,16g+j] + b[16f+i]
i.e. per (f,g) 16x16 block the weight is circulant. Computed in the rfft
domain: for each of the 9 rfft bins k, Yhat_k[B,f] = sum_g Phat_k[f,g] *
Xhat_k[B,g] (complex). The cheap length-16 rfft/irfft transforms run on the
host; the einsum over g runs on 8 NeuronCores (data-parallel over the batch).

Per component pair p (p=0 holds the two real bins 0 and 8; p=1..7 holds
(Re_p, Im_p)) the device computes, with K=128 g on partitions:
  p=0 : Y0 = W0^T X0 ; Y8 = W8^T X8
  p>=1: Yr = Wr^T Xr - Wi^T Xi ; Yi = Wr^T Xi + Wi^T Xr
as 4 accumulating [128,128]@[128,512] matmuls per pair (2 for p=0), where
the negated weight tile (-Wi) is built on device by one DVE negate per pair
so only the information-tight weight stream (pr,pi per pair = 512KB fp16)
ever crosses HBM.

Data budget per core and iteration (the DMA floor is the bottleneck):
  x   : rfft comps, fp8 E3M4, per-pair host scales  1.0 MB
  w   : fp16                                        0.5 MB
  out : fp16 (host applies inverse scales + irfft)  2.0 MB
fp8 E3M4 keeps 4 mantissa bits; with fp16 weights and fp32 PSUM accumulation
the end-to-end max rel err is ~1.2e-2 (budget 2e-2; fp16 x gives 3.6e-4).

Schedule notes (from TimelineSim iteration): input DMAs issue on the SP
HWDGE queue, output DMAs on the ACT queue — a queued store waiting for its
drain must not block the next iteration's loads. The bench loop unrolls 2
bodies per For_i iteration so the staggered-reset boundary sync amortizes
and the PE p-state stays hot. PSUM comp banks drain on DVE and ACT in
parallel so stores start sooner.
"""

import numpy as np
import ml_dtypes

import concourse.mybir as mybir
import concourse.tile as tile
from concourse import bacc
from concourse.bass_utils import run_bass_kernel_spmd

N_CORES = 8
BATCH = 4096
IN_FEATURES = 2048
OUT_FEATURES = 2048
BS = 16          # circulant block size
NB = 128         # feature blocks (f and g)
BINS = 9         # rfft bins of length-16 signal
NPAIR = 8        # component pairs: (re0,re8), (re1,im1), ..., (re7,im7)
BC = BATCH // N_CORES  # 512 batch rows per core

# interleaved input DMA issue order: (kind, first pair, npairs)
ICHUNKS = [("w", 0, 2), ("x", 0, 1), ("x", 1, 1), ("w", 2, 2), ("x", 2, 2),
           ("w", 4, 4), ("x", 4, 2), ("x", 6, 2)]
OCHUNKS = [(0, 2), (2, 2), (4, 2), (6, 2)]  # (first pair, npairs) per store

UNROLL = 2       # bodies per For_i iteration in the bench-loop variant
SPLIT_DRAIN = True           # drain comp banks on DVE/ACT in parallel
OENG = lambda nc: nc.scalar  # engine issuing output DMAs
IENG = lambda nc: nc.sync    # engine issuing input DMAs

XDT = mybir.dt.float8e3   # x stream dtype
WDT = mybir.dt.float16    # weight stream dtype
ODT = mybir.dt.float16    # output stream dtype
FP8_MAX = 15.0            # quantization target max (E3M4 max finite 15.5)

_DT_NP = {
    mybir.dt.float32: np.float32,
    mybir.dt.bfloat16: ml_dtypes.bfloat16,
    mybir.dt.float16: np.float16,
    mybir.dt.float8e3: ml_dtypes.float8_e3m4,
}

_CACHED = {}
NWARM = 8        # dummy PE-warmup matmuls issued during the initial DMA wait
                 # (PE p-state needs ~3.4us of accumulated busy to unthrottle)


def _emit_body(nc, tc, pools, xin, win, yout, warm=0):
    f32 = mybir.dt.float32
    xp, wp, wm, op, ps = pools
    xts = [None] * NPAIR   # per pair: (tile, idx within tile)
    wts = [None] * NPAIR
    for kind, p0, n in ICHUNKS:
        if kind == "x":
            t = xp.tile([128, n, 2, BC], XDT, tag=f"x{p0}")
            IENG(nc).dma_start(t[:], xin[:, p0:p0 + n])
            for i in range(n):
                xts[p0 + i] = (t, i)
        else:
            t = wp.tile([128, n, 2, 128], WDT, tag=f"w{p0}")
            IENG(nc).dma_start(t[:], win[:, p0:p0 + n])
            for i in range(n):
                wts[p0 + i] = (t, i)
    if warm:
        # dummy matmuls on a zeroed scratch tile accumulate PE-busy while the
        # first input DMAs are in flight, so real matmuls run at 2.4GHz
        z = xp.tile([128, 512], XDT, tag="warmz")
        nc.gpsimd.memset(z[:], 0.0)
        wps = tc.warm_pool.tile([128, 512], f32, tag="warmp")
        for _ in range(warm):
            nc.tensor.matmul(wps[:], z[:, :128], z[:], start=True, stop=True)
    # Negated Wi tiles (-Im Phat), one DVE negate per complex pair.
    wmt = wm.tile([128, NPAIR - 1, 128], WDT, tag="wm")
    for p in range(1, NPAIR):
        wtile, wi = wts[p]
        nc.vector.tensor_scalar_mul(wmt[:, p - 1], wtile[:, wi, 1], -1.0)
    for ci, (p0, n) in enumerate(OCHUNKS):
        oc = op.tile([128, n, 2, BC], ODT, tag=f"o{p0}")
        for pp in range(n):
            p = p0 + pp
            xtile, xi = xts[p]
            wtile, wi = wts[p]
            xt = xtile[:, xi]                  # [128, 2, BC]   fp8
            wt = wtile[:, wi]                  # [128, 2, 128]  fp16
            acc = ps.tile([128, 2, BC], f32, tag="acc")
            if p == 0:
                nc.tensor.matmul(acc[:, 0], wt[:, 0], xt[:, 0],
                                 start=True, stop=True)
                nc.tensor.matmul(acc[:, 1], wt[:, 1], xt[:, 1],
                                 start=True, stop=True)
            else:
                # Yr = Wr X_r + (-Wi) X_i ; Yi = Wr X_i + Wi X_r
                nc.tensor.matmul(acc[:, 0], wt[:, 0], xt[:, 0],
                                 start=True, stop=False)
                nc.tensor.matmul(acc[:, 0], wmt[:, p - 1], xt[:, 1],
                                 start=False, stop=True)
                nc.tensor.matmul(acc[:, 1], wt[:, 0], xt[:, 1],
                                 start=True, stop=False)
                nc.tensor.matmul(acc[:, 1], wt[:, 1], xt[:, 0],
                                 start=False, stop=True)
            if SPLIT_DRAIN:
                nc.vector.tensor_copy(out=oc[:, pp, 0], in_=acc[:, 0])
                nc.scalar.copy(out=oc[:, pp, 1], in_=acc[:, 1])
            elif p % 2 == 0:
                nc.vector.tensor_copy(out=oc[:, pp], in_=acc[:])
            else:
                nc.scalar.copy(out=oc[:, pp], in_=acc[:])
        OENG(nc).dma_start(yout[:, p0:p0 + n], oc[:])


def _build_nc(loop_reps=0):
    """Build the Bass program (one NEFF, SPMD across 8 cores).

    loop_reps > 0 wraps the body in a For_i loop running it that many times
    (benchmarking variant; output identical since iterations are idempotent).
    """
    nc = bacc.Bacc("TRN2", target_bir_lowering=False, num_devices=N_CORES)
    xin = nc.dram_tensor("xin", [128, NPAIR, 2, BC], XDT,
                         kind="ExternalInput")
    win = nc.dram_tensor("win", [128, NPAIR, 2, 128], WDT,
                         kind="ExternalInput")
    yout = nc.dram_tensor("yout", [128, NPAIR, 2, BC], ODT,
                          kind="ExternalOutput")

    with tile.TileContext(nc) as tc:
        import contextlib
        with (
            tc.tile_pool(name="xp", bufs=2) as xp,
            tc.tile_pool(name="wp", bufs=2) as wp,
            tc.tile_pool(name="wm", bufs=2) as wm,
            tc.tile_pool(name="op", bufs=2) as op,
            tc.tile_pool(name="ps", bufs=4 if loop_reps else 3,
                         space="PSUM") as ps,
            (contextlib.nullcontext() if loop_reps else
             tc.tile_pool(name="warmps", bufs=1, space="PSUM")) as warm_pool,
        ):
            tc.warm_pool = warm_pool
            pools = (xp, wp, wm, op, ps)
            if loop_reps:
                unroll = next(u for u in (UNROLL, 2, 1) if loop_reps % u == 0)
                with tc.For_i(0, loop_reps // unroll, 1, staggered_reset=True):
                    for _ in range(unroll):
                        _emit_body(nc, tc, pools, xin, win, yout)
            else:
                _emit_body(nc, tc, pools, xin, win, yout, warm=NWARM)
    nc.compile()
    return nc


def _host_prep_weights(index_vectors):
    """win[g, p, c, f] fp16: per pair the two stationary tiles
    (p=0: bin0/bin8 real; p>=1: Re_p, Im_p), transposed to [g, f]."""
    Pf = np.fft.rfft(np.asarray(index_vectors).astype(np.float64), axis=-1)
    win = np.empty((128, NPAIR, 2, 128), dtype=np.float64)  # [g, p, c, f]
    win[:, 0, 0] = Pf[:, :, 0].real.T
    win[:, 0, 1] = Pf[:, :, 8].real.T
    for p in range(1, NPAIR):
        win[:, p, 0] = Pf[:, :, p].real.T
        win[:, p, 1] = Pf[:, :, p].imag.T
    return np.ascontiguousarray(win.astype(_DT_NP[WDT]))


def _host_prep_x(x):
    """xin[core][g, p, c, b] fp8 + per-pair scales."""
    Xf = np.fft.rfft(np.asarray(x).reshape(BATCH, NB, BS), axis=-1)
    XfT = Xf.transpose(1, 2, 0)  # (g, bin, B)
    comps = np.empty((NPAIR, 2, NB, BATCH), dtype=np.float64)
    comps[0, 0] = XfT[:, 0].real
    comps[0, 1] = XfT[:, 8].real
    for p in range(1, NPAIR):
        comps[p, 0] = XfT[:, p].real
        comps[p, 1] = XfT[:, p].imag
    scales = np.abs(comps).reshape(NPAIR, -1).max(axis=1) / FP8_MAX
    q = (comps / scales[:, None, None, None]).astype(np.float32)
    q = q.astype(_DT_NP[XDT])                      # (p, c, g, B)
    q = q.reshape(NPAIR, 2, NB, N_CORES, BC)
    q = q.transpose(3, 2, 0, 1, 4)                 # (core, g, p, c, b)
    return np.ascontiguousarray(q), scales


def _host_post(youts, b, scales):
    """Reassemble Yhat bins from the 8 cores' outputs, irfft, add bias."""
    Yf = np.empty((BATCH, NB, BINS), dtype=np.complex128)
    y = np.stack([np.asarray(yc) for yc in youts])  # [core, f, p, c, b]
    y = y.astype(np.float64) * scales.reshape(1, 1, NPAIR, 1, 1)
    y = y.transpose(0, 2, 3, 4, 1)                  # [core, p, c, b, f]
    for core in range(N_CORES):
        bsl = slice(core * BC, (core + 1) * BC)
        yc = y[core]
        Yf[bsl, :, 0] = yc[0, 0]
        Yf[bsl, :, 8] = yc[0, 1]
        for p in range(1, NPAIR):
            Yf[bsl, :, p] = yc[p, 0] + 1j * yc[p, 1]
    out = np.fft.irfft(Yf, n=BS, axis=-1).reshape(BATCH, OUT_FEATURES)
    return (out + np.asarray(b).astype(np.float64)).astype(np.float32)


def run(x, index_vectors, b, trace=False):
    if "nc" not in _CACHED:
        _CACHED["nc"] = _build_nc()
    nc = _CACHED["nc"]
    win = _host_prep_weights(index_vectors)
    xin, scales = _host_prep_x(x)
    in_maps = [{"xin": xin[c], "win": win} for c in range(N_CORES)]
    res = run_bass_kernel_spmd(nc, in_maps, core_ids=list(range(N_CORES)),
                               trace=trace)
    youts = [res.results[c]["yout"] for c in range(N_CORES)]
    out = _host_post(youts, b, scales)
    return out, res


def kernel(x, index_vectors, b):
    out, _ = run(x, index_vectors, b)
    return out


# revision 17
# speedup vs baseline: 2.9831x; 2.9831x over previous
"""Trainium2 Bass kernel for the EnhancedBCMLayer (block-circulant matrix layer).

Math: out[B, 16f+i] = sum_{g,j} iv[f,g,(i-j)%16] * x[B,16g+j] + b[16f+i]
i.e. per (f,g) 16x16 block the weight is circulant. Computed in the rfft
domain: for each of the 9 rfft bins k, Yhat_k[B,f] = sum_g Phat_k[f,g] *
Xhat_k[B,g] (complex). The cheap length-16 rfft/irfft transforms run on the
host; the einsum over g runs on 8 NeuronCores (data-parallel over the batch).

Per component pair p (p=0 holds the two real bins 0 and 8; p=1..7 holds
(Re_p, Im_p)) the device computes, with K=128 g on partitions:
  p=0 : Y0 = W0^T X0 ; Y8 = W8^T X8
  p>=1: Yr = Wr^T Xr - Wi^T Xi ; Yi = Wr^T Xi + Wi^T Xr
as 4 accumulating [128,128]@[128,512] matmuls per pair (2 for p=0), where
the negated weight tile (-Wi) is built on device by one DVE negate per pair
so only the information-tight weight stream (pr,pi per pair = 512KB fp16)
ever crosses HBM.

Data budget per core and iteration (the DMA floor is the bottleneck):
  x   : rfft comps, fp8 E3M4, per-pair host scales  1.0 MB
  w   : fp16                                        0.5 MB
  out : fp16 (host applies inverse scales + irfft)  2.0 MB
fp8 E3M4 keeps 4 mantissa bits; with fp16 weights and fp32 PSUM accumulation
the end-to-end max rel err is ~1.2e-2 (budget 2e-2; fp16 x gives 3.6e-4).

Schedule notes (from TimelineSim iteration): input DMAs issue on the SP
HWDGE queue, output DMAs on the ACT queue — a queued store waiting for its
drain must not block the next iteration's loads. The bench loop unrolls 2
bodies per For_i iteration so the staggered-reset boundary sync amortizes
and the PE p-state stays hot. PSUM comp banks drain on DVE and ACT in
parallel so stores start sooner.
"""

import numpy as np
import ml_dtypes

import concourse.mybir as mybir
import concourse.tile as tile
from concourse import bacc
from concourse.bass_utils import run_bass_kernel_spmd

N_CORES = 8
BATCH = 4096
IN_FEATURES = 2048
OUT_FEATURES = 2048
BS = 16          # circulant block size
NB = 128         # feature blocks (f and g)
BINS = 9         # rfft bins of length-16 signal
NPAIR = 8        # component pairs: (re0,re8), (re1,im1), ..., (re7,im7)
BC = BATCH // N_CORES  # 512 batch rows per core

# interleaved input DMA issue order: (kind, first pair, npairs).
# Few, large DMAs: HW charges ~570ns of serial overhead per DMA on top of
# bytes/435GB/s, so 3 loads + 4 stores beats fine-grained chunking.
ICHUNKS = [("w", 0, 8), ("x", 0, 4), ("x", 4, 4)]
OCHUNKS = [(0, 2), (2, 2), (4, 2), (6, 2)]  # (first pair, npairs) per store

UNROLL = 4       # bodies per For_i iteration in the bench-loop variant
BUFS = 4         # SBUF tile pool rotation depth (cross-iteration overlap)
SPLIT_DRAIN = True           # drain comp banks on DVE/ACT in parallel
OENG = lambda nc: nc.scalar  # engine issuing output DMAs
IENG = lambda nc: nc.sync    # engine issuing input DMAs

XDT = mybir.dt.float8e3   # x stream dtype
WDT = mybir.dt.float16    # weight stream dtype
ODT = mybir.dt.float16    # output stream dtype
FP8_MAX = 15.0            # quantization target max (E3M4 max finite 15.5)

_DT_NP = {
    mybir.dt.float32: np.float32,
    mybir.dt.bfloat16: ml_dtypes.bfloat16,
    mybir.dt.float16: np.float16,
    mybir.dt.float8e3: ml_dtypes.float8_e3m4,
}

_CACHED = {}
NWARM = 8        # dummy PE-warmup matmuls issued during the initial DMA wait
                 # (PE p-state needs ~3.4us of accumulated busy to unthrottle)


def _emit_body(nc, tc, pools, xin, win, yout, warm=0):
    f32 = mybir.dt.float32
    xp, wp, wm, op, ps = pools
    xts = [None] * NPAIR   # per pair: (tile, idx within tile)
    wts = [None] * NPAIR
    for kind, p0, n in ICHUNKS:
        if kind == "x":
            t = xp.tile([128, n, 2, BC], XDT, tag=f"x{p0}")
            IENG(nc).dma_start(t[:], xin[:, p0:p0 + n])
            for i in range(n):
                xts[p0 + i] = (t, i)
        else:
            t = wp.tile([128, n, 2, 128], WDT, tag=f"w{p0}")
            IENG(nc).dma_start(t[:], win[:, p0:p0 + n])
            for i in range(n):
                wts[p0 + i] = (t, i)
    if warm:
        # dummy matmuls on a zeroed scratch tile accumulate PE-busy while the
        # first input DMAs are in flight, so real matmuls run at 2.4GHz
        z = xp.tile([128, 512], XDT, tag="warmz")
        nc.gpsimd.memset(z[:], 0.0)
        wps = tc.warm_pool.tile([128, 512], f32, tag="warmp")
        for _ in range(warm):
            nc.tensor.matmul(wps[:], z[:, :128], z[:], start=True, stop=True)
    # Negated Wi tiles (-Im Phat), one DVE negate per complex pair.
    wmt = wm.tile([128, NPAIR - 1, 128], WDT, tag="wm")
    for p in range(1, NPAIR):
        wtile, wi = wts[p]
        nc.vector.tensor_scalar_mul(wmt[:, p - 1], wtile[:, wi, 1], -1.0)
    for ci, (p0, n) in enumerate(OCHUNKS):
        oc = op.tile([128, n, 2, BC], ODT, tag=f"o{p0}")
        for pp in range(n):
            p = p0 + pp
            xtile, xi = xts[p]
            wtile, wi = wts[p]
            xt = xtile[:, xi]                  # [128, 2, BC]   fp8
            wt = wtile[:, wi]                  # [128, 2, 128]  fp16
            acc = ps.tile([128, 2, BC], f32, tag="acc")
            if p == 0:
                nc.tensor.matmul(acc[:, 0], wt[:, 0], xt[:, 0],
                                 start=True, stop=True)
                nc.tensor.matmul(acc[:, 1], wt[:, 1], xt[:, 1],
                                 start=True, stop=True)
            else:
                # Yr = Wr X_r + (-Wi) X_i ; Yi = Wr X_i + Wi X_r
                nc.tensor.matmul(acc[:, 0], wt[:, 0], xt[:, 0],
                                 start=True, stop=False)
                nc.tensor.matmul(acc[:, 0], wmt[:, p - 1], xt[:, 1],
                                 start=False, stop=True)
                nc.tensor.matmul(acc[:, 1], wt[:, 0], xt[:, 1],
                                 start=True, stop=False)
                nc.tensor.matmul(acc[:, 1], wt[:, 1], xt[:, 0],
                                 start=False, stop=True)
            if SPLIT_DRAIN:
                nc.vector.tensor_copy(out=oc[:, pp, 0], in_=acc[:, 0])
                nc.scalar.copy(out=oc[:, pp, 1], in_=acc[:, 1])
            elif p % 2 == 0:
                nc.vector.tensor_copy(out=oc[:, pp], in_=acc[:])
            else:
                nc.scalar.copy(out=oc[:, pp], in_=acc[:])
        OENG(nc).dma_start(yout[:, p0:p0 + n], oc[:])


def _build_nc(loop_reps=0):
    """Build the Bass program (one NEFF, SPMD across 8 cores).

    loop_reps > 0 wraps the body in a For_i loop running it that many times
    (benchmarking variant; output identical since iterations are idempotent).
    """
    nc = bacc.Bacc("TRN2", target_bir_lowering=False, num_devices=N_CORES)
    xin = nc.dram_tensor("xin", [128, NPAIR, 2, BC], XDT,
                         kind="ExternalInput")
    win = nc.dram_tensor("win", [128, NPAIR, 2, 128], WDT,
                         kind="ExternalInput")
    yout = nc.dram_tensor("yout", [128, NPAIR, 2, BC], ODT,
                          kind="ExternalOutput")

    with tile.TileContext(nc) as tc:
        import contextlib
        with (
            tc.tile_pool(name="xp", bufs=BUFS) as xp,
            tc.tile_pool(name="wp", bufs=BUFS) as wp,
            tc.tile_pool(name="wm", bufs=BUFS) as wm,
            tc.tile_pool(name="op", bufs=BUFS) as op,
            tc.tile_pool(name="ps", bufs=4 if loop_reps else 3,
                         space="PSUM") as ps,
            (contextlib.nullcontext() if loop_reps else
             tc.tile_pool(name="warmps", bufs=1, space="PSUM")) as warm_pool,
        ):
            tc.warm_pool = warm_pool
            pools = (xp, wp, wm, op, ps)
            if loop_reps:
                unroll = next(u for u in (UNROLL, 2, 1) if loop_reps % u == 0)
                with tc.For_i(0, loop_reps // unroll, 1, staggered_reset=True):
                    for _ in range(unroll):
                        _emit_body(nc, tc, pools, xin, win, yout)
            else:
                _emit_body(nc, tc, pools, xin, win, yout, warm=NWARM)
    nc.compile()
    return nc


def _host_prep_weights(index_vectors):
    """win[g, p, c, f] fp16: per pair the two stationary tiles
    (p=0: bin0/bin8 real; p>=1: Re_p, Im_p), transposed to [g, f]."""
    Pf = np.fft.rfft(np.asarray(index_vectors).astype(np.float64), axis=-1)
    win = np.empty((128, NPAIR, 2, 128), dtype=np.float64)  # [g, p, c, f]
    win[:, 0, 0] = Pf[:, :, 0].real.T
    win[:, 0, 1] = Pf[:, :, 8].real.T
    for p in range(1, NPAIR):
        win[:, p, 0] = Pf[:, :, p].real.T
        win[:, p, 1] = Pf[:, :, p].imag.T
    return np.ascontiguousarray(win.astype(_DT_NP[WDT]))


def _host_prep_x(x):
    """xin[core][g, p, c, b] fp8 + per-pair scales."""
    Xf = np.fft.rfft(np.asarray(x).reshape(BATCH, NB, BS), axis=-1)
    XfT = Xf.transpose(1, 2, 0)  # (g, bin, B)
    comps = np.empty((NPAIR, 2, NB, BATCH), dtype=np.float64)
    comps[0, 0] = XfT[:, 0].real
    comps[0, 1] = XfT[:, 8].real
    for p in range(1, NPAIR):
        comps[p, 0] = XfT[:, p].real
        comps[p, 1] = XfT[:, p].imag
    scales = np.abs(comps).reshape(NPAIR, -1).max(axis=1) / FP8_MAX
    q = (comps / scales[:, None, None, None]).astype(np.float32)
    q = q.astype(_DT_NP[XDT])                      # (p, c, g, B)
    q = q.reshape(NPAIR, 2, NB, N_CORES, BC)
    q = q.transpose(3, 2, 0, 1, 4)                 # (core, g, p, c, b)
    return np.ascontiguousarray(q), scales


def _host_post(youts, b, scales):
    """Reassemble Yhat bins from the 8 cores' outputs, irfft, add bias."""
    Yf = np.empty((BATCH, NB, BINS), dtype=np.complex128)
    y = np.stack([np.asarray(yc) for yc in youts])  # [core, f, p, c, b]
    y = y.astype(np.float64) * scales.reshape(1, 1, NPAIR, 1, 1)
    y = y.transpose(0, 2, 3, 4, 1)                  # [core, p, c, b, f]
    for core in range(N_CORES):
        bsl = slice(core * BC, (core + 1) * BC)
        yc = y[core]
        Yf[bsl, :, 0] = yc[0, 0]
        Yf[bsl, :, 8] = yc[0, 1]
        for p in range(1, NPAIR):
            Yf[bsl, :, p] = yc[p, 0] + 1j * yc[p, 1]
    out = np.fft.irfft(Yf, n=BS, axis=-1).reshape(BATCH, OUT_FEATURES)
    return (out + np.asarray(b).astype(np.float64)).astype(np.float32)


def run(x, index_vectors, b, trace=False):
    if "nc" not in _CACHED:
        _CACHED["nc"] = _build_nc()
    nc = _CACHED["nc"]
    win = _host_prep_weights(index_vectors)
    xin, scales = _host_prep_x(x)
    in_maps = [{"xin": xin[c], "win": win} for c in range(N_CORES)]
    res = run_bass_kernel_spmd(nc, in_maps, core_ids=list(range(N_CORES)),
                               trace=trace)
    youts = [res.results[c]["yout"] for c in range(N_CORES)]
    out = _host_post(youts, b, scales)
    return out, res


def kernel(x, index_vectors, b):
    out, _ = run(x, index_vectors, b)
    return out
